# revision 1
# baseline (speedup 1.0000x reference)
"""DGCNN classifier Trainium2 kernel (Bass/Tile), data-parallel over batch on 8 cores.

Reformulation (per EdgeConv layer with weight W=[Wd|Wc], BN scale g, bias b):
    pre(n,j) = W @ [h_j - h_n; h_n] = Wd h_j + (Wc-Wd) h_n
    BN scale s>0 and LReLU are monotone, so they commute with the neighbor max:
    h'(n) = LReLU( max_{j in knn(n)} (s*Wd h_j) + (s*(Wc-Wd) h_n + b) )
Per layer: pairwise -dist^2 matrix D from augmented PE matmuls, exact top-20
per row via DVE max8/max_index/match_replace, gather of uT = (s*Wd) h
(channel-major) via gpsimd ap_gather from SBUF, neighbor max via DVE strided
tensor_reduce, +vT and LReLU on ACT. Features stay channel-major [C, N]
throughout (no transposes; the four layer outputs are conv5's K-tiles).
"""

import numpy as np

K = 20
EPS = 1e-5
NEG = 0.2
N = 1024
B = 16
NCORES = 8
SPC = B // NCORES  # samples per core
CS = [3, 64, 64, 128]      # in-channels per edgeconv layer
OS = [64, 64, 128, 256]    # out-channels per edgeconv layer
NEGINF = -1.0e30

_cache = {}


def _build_program():
    import concourse.mybir as mybir
    from concourse import bacc
    import concourse.tile as tile

    f32 = mybir.dt.float32
    u32 = mybir.dt.uint32
    i16 = mybir.dt.int16
    AF = mybir.ActivationFunctionType
    ALU = mybir.AluOpType
    AX = mybir.AxisListType

    nc = bacc.Bacc("TRN2", target_bir_lowering=False, debug=False,
                   enable_asserts=False, num_devices=NCORES)

    # ---- DRAM I/O -------------------------------------------------------
    xt_d = nc.dram_tensor("xt", (SPC, 3, N), f32, kind="ExternalInput")
    ru_d = [nc.dram_tensor(f"ru{i+1}", (CS[i], OS[i]), f32, kind="ExternalInput")
            for i in range(4)]
    rvw_d = [nc.dram_tensor(f"rvw{i+1}", (CS[i], OS[i]), f32, kind="ExternalInput")
             for i in range(4)]
    rvb_d = [nc.dram_tensor(f"rvb{i+1}", (1, OS[i]), f32, kind="ExternalInput")
             for i in range(4)]
    w5_d = nc.dram_tensor("w5t", (513, N), f32, kind="ExternalInput")
    l1_d = nc.dram_tensor("l1t", (2048, 512), f32, kind="ExternalInput")
    b6_d = nc.dram_tensor("b6v", (128, 4), f32, kind="ExternalInput")
    l2_d = nc.dram_tensor("l2t", (512, 256), f32, kind="ExternalInput")
    b7_d = nc.dram_tensor("b7v", (128, 2), f32, kind="ExternalInput")
    l3_d = nc.dram_tensor("l3t", (256, 40), f32, kind="ExternalInput")
    b3_d = nc.dram_tensor("b3v", (40, 1), f32, kind="ExternalInput")
    axsb_d = nc.dram_tensor("axsb", (2, 2), f32, kind="ExternalInput")
    rxsb_d = nc.dram_tensor("rxsb", (2, 2), f32, kind="ExternalInput")
    idf_d = nc.dram_tensor("idf", (128, 128), f32, kind="ExternalInput")
    out_d = nc.dram_tensor("out", (SPC, 40), f32, kind="ExternalOutput")

    with tile.TileContext(nc) as tc:
        cpool = tc.alloc_tile_pool(name="const", bufs=1)
        hpool = tc.alloc_tile_pool(name="hts", bufs=1)
        dpool = tc.alloc_tile_pool(name="dsb", bufs=2)
        wpool = tc.alloc_tile_pool(name="work", bufs=2)
        gpool = tc.alloc_tile_pool(name="gath", bufs=2)
        dram_pool = tc.alloc_tile_pool(name="wdram", bufs=1, space="DRAM")
        ps_d = tc.alloc_tile_pool(name="ps_d", bufs=2, space="PSUM")
        ps_m = tc.alloc_tile_pool(name="ps_m", bufs=2, space="PSUM")
        ps_t = tc.alloc_tile_pool(name="ps_t", bufs=2, space="PSUM")

        def load_const(ap, tag):
            t = cpool.tile(list(ap.shape), ap.dtype, tag=tag)
            nc.sync.dma_start(t[:], ap[:])
            return t

        ru = [load_const(ru_d[i].ap(), f"ru{i}") for i in range(4)]
        rvw = [load_const(rvw_d[i].ap(), f"rvw{i}") for i in range(4)]
        rvb = [load_const(rvb_d[i].ap(), f"rvb{i}") for i in range(4)]
        idf = load_const(idf_d.ap(), "idf")
        b6 = load_const(b6_d.ap(), "b6")
        b7 = load_const(b7_d.ap(), "b7")
        b3 = load_const(b3_d.ap(), "b3")
        axsb = load_const(axsb_d.ap(), "axsb")
        rxsb = load_const(rxsb_d.ap(), "rxsb")
        # w5t split into K-tiles matching [h1,h2,h3,h4a,h4b,ones]
        w5sb = []
        w5rows = [(0, 64), (64, 128), (128, 256), (256, 384), (384, 512), (512, 513)]
        for i, (r0, r1) in enumerate(w5rows):
            t = cpool.tile([r1 - r0, N], f32, tag=f"w5_{i}")
            nc.sync.dma_start(t[:], w5_d.ap()[r0:r1, :])
            w5sb.append(t)
        l2sb = []
        for kk in range(4):
            t = cpool.tile([128, 256], f32, tag=f"l2_{kk}")
            nc.sync.dma_start(t[:], l2_d.ap()[kk * 128:(kk + 1) * 128, :])
            l2sb.append(t)
        l3sb = []
        for kk in range(2):
            t = cpool.tile([128, 40], f32, tag=f"l3_{kk}")
            nc.sync.dma_start(t[:], l3_d.ap()[kk * 128:(kk + 1) * 128, :])
            l3sb.append(t)

        onescol = cpool.tile([128, 2], f32, tag="onescol")
        nc.gpsimd.memset(onescol[:], 1.0)
        onesrow = cpool.tile([1, N], f32, tag="onesrow")
        nc.gpsimd.memset(onesrow[:], 1.0)

        pooled2 = cpool.tile([128, 16, SPC], f32, tag="pooled2")
        maxs = cpool.tile([128, 16, SPC], f32, tag="maxs")
        sums = cpool.tile([128, 16, SPC], f32, tag="sums")

        def edgeconv(s, L, hT):
            """EdgeConv layer, channel-major. hT: [C, N] tile. Returns list of
            <=128-row [*, N] tiles holding the transposed output features."""
            C, O = CS[L], OS[L]
            nblk = (O + 127) // 128
            # ---- sq(n) = sum_c hT(c,n)^2, duplicated on partitions 0-1
            h2 = wpool.tile([C, N], f32, tag="h2sq")
            nc.scalar.activation(h2[:], hT[0:C, :], AF.Square)
            sqps = []
            for half in range(2):
                sqp = ps_m.tile([128, 512], f32, tag="mm")
                nc.tensor.matmul(sqp[0:2, :], lhsT=onescol[0:C, 0:2],
                                 rhs=h2[0:C, half * 512:(half + 1) * 512],
                                 start=True, stop=True)
                sqps.append(sqp)
            # ---- A = [2hT; -sq; -1], R = [hT; 1; sq]
            Ah = wpool.tile([C, N], f32, tag="Ah")
            nc.scalar.activation(Ah[:], hT[0:C, :], AF.Copy, scale=2.0)
            Ax = wpool.tile([2, N], f32, tag="Ax")
            Rx = wpool.tile([2, N], f32, tag="Rx")
            for half in range(2):
                nsl = slice(half * 512, (half + 1) * 512)
                nc.scalar.activation(Ax[0:2, nsl], sqps[half][0:2, :], AF.Identity,
                                     scale=axsb[:, 0:1], bias=axsb[:, 1:2])
                nc.scalar.activation(Rx[0:2, nsl], sqps[half][0:2, :], AF.Identity,
                                     scale=rxsb[:, 0:1], bias=rxsb[:, 1:2])
            # ---- uT, vT (channel-major)
            uts, vts = [], []
            for ot in range(nblk):
                oc = min(128, O - ot * 128)
                osl = slice(ot * 128, ot * 128 + oc)
                ut = wpool.tile([128, N], f32, tag=f"ut{ot}")
                vt = wpool.tile([128, N], f32, tag=f"vt{ot}")
                for half in range(2):
                    nsl = slice(half * 512, (half + 1) * 512)
                    up = ps_m.tile([128, 512], f32, tag="mm")
                    nc.tensor.matmul(up[0:oc, :], lhsT=ru[L][:, osl],
                                     rhs=hT[0:C, nsl], start=True, stop=True)
                    nc.scalar.copy(ut[0:oc, nsl], up[0:oc, :])
                    vp = ps_m.tile([128, 512], f32, tag="mm")
                    nc.tensor.matmul(vp[0:oc, :], lhsT=rvw[L][:, osl],
                                     rhs=hT[0:C, nsl], start=True, stop=False)
                    nc.tensor.matmul(vp[0:oc, :], lhsT=rvb[L][:, osl],
                                     rhs=onesrow[:, nsl], start=False, stop=True)
                    nc.scalar.copy(vt[0:oc, nsl], vp[0:oc, :])
                uts.append(ut)
                vts.append(vt)

            hT_new = []
            for ot in range(nblk):
                hT_new.append(hpool.tile([128, N], f32, tag=f"hT_{s}_{L}_{ot}", name=f"hT_{s}_{L}_{ot}"))

            # ---- per-block: D, top-20, marshal wrapped list into W_all
            wall = wpool.tile([16, 8 * 160], i16, tag="wall")
            for b in range(8):
                msl = slice(b * 128, (b + 1) * 128)
                Dp = ps_d.tile([128, N], f32, tag="Dp")
                for half in range(2):
                    nsl = slice(half * 512, (half + 1) * 512)
                    nc.tensor.matmul(Dp[:, nsl], lhsT=Ah[0:C, msl],
                                     rhs=hT[0:C, nsl], start=True, stop=False)
                    nc.tensor.matmul(Dp[:, nsl], lhsT=Ax[:, msl],
                                     rhs=Rx[:, nsl], start=False, stop=True)
                Dsb = dpool.tile([128, N], f32, tag="Dsb")
                vals = wpool.tile([128, 24], f32, tag="vals")
                idx = wpool.tile([128, 24], u32, tag="idx")
                nc.vector.max(vals[:, 0:8], Dp[:])
                nc.vector.max_index(idx[:, 0:8], vals[:, 0:8], Dp[:])
                nc.vector.match_replace(Dsb[:], vals[:, 0:8], Dp[:], NEGINF)
                nc.vector.max(vals[:, 8:16], Dsb[:])
                nc.vector.max_index(idx[:, 8:16], vals[:, 8:16], Dsb[:])
                nc.vector.match_replace(Dsb[:], vals[:, 8:16], Dsb[:], NEGINF)
                nc.vector.max(vals[:, 16:24], Dsb[:])
                nc.vector.max_index(idx[:, 16:24], vals[:, 16:24], Dsb[:])
                # marshal: wrapped list W[q, s*8+pp] = idx[16*pp+q, s]
                idxf = wpool.tile([128, 24], f32, tag="idxf")
                nc.vector.tensor_copy(idxf[:], idx[:])
                t1p = ps_t.tile([24, 128], f32, tag="tp")
                nc.tensor.transpose(t1p[:], idxf[:], idf[:])
                t1s = wpool.tile([24, 128], f32, tag="t1s")
                nc.scalar.copy(t1s[:], t1p[:])
                wp = ps_t.tile([16, 160], f32, tag="tp")
                for pp in range(8):
                    nc.tensor.transpose(wp[0:16, pp * 20:(pp + 1) * 20],
                                        t1s[0:20, pp * 16:(pp + 1) * 16],
                                        idf[0:20, 0:20])
                nc.scalar.copy(
                    wall[:, b * 160:(b + 1) * 160].rearrange(
                        "q (s pp) -> q pp s", s=20),
                    wp[:].rearrange("q (pp s) -> q pp s", pp=8))

            # ---- replicate wrapped lists to all 8 core groups (DRAM bounce)
            wd = dram_pool.tile([16, 8 * 160], i16, tag="wd")
            nc.sync.dma_start(wd[:], wall[:])
            wrep = wpool.tile([128, 8 * 160], i16, tag="wrep")
            for r8 in range(8):
                nc.sync.dma_start(wrep[r8 * 16:(r8 + 1) * 16, :], wd[:])

            # ---- gather + neighbor-max + vT + LReLU
            for b in range(8):
                msl = slice(b * 128, (b + 1) * 128)
                for ot in range(nblk):
                    oc = min(128, O - ot * 128)
                    g = gpool.tile([128, 2560], f32, tag="g")
                    nc.gpsimd.ap_gather(
                        out_ap=g[0:oc, :], in_ap=uts[ot][0:oc, :],
                        idxs_ap=wrep[0:oc, b * 160:(b + 1) * 160],
                        channels=oc, num_elems=N, d=1, num_idxs=2560)
                    acc = wpool.tile([128, 128], f32, tag="acc")
                    nc.vector.tensor_reduce(
                        acc[0:oc, :],
                        g[0:oc, :].rearrange("o (s p) -> o p s", s=20),
                        axis=AX.X, op=ALU.max)
                    nc.vector.tensor_add(acc[0:oc, :], acc[0:oc, :],
                                         vts[ot][0:oc, msl])
                    nc.scalar.activation(hT_new[ot][0:oc, msl], acc[0:oc, :],
                                         AF.Prelu, alpha=NEG)
            return hT_new

        for s in range(SPC):
            x_sb = hpool.tile([3, N], f32, tag=f"x_{s}")
            nc.sync.dma_start(x_sb[:], xt_d.ap()[s])
            h1 = edgeconv(s, 0, x_sb)          # [64,N]
            h2t = edgeconv(s, 1, h1[0])        # [64,N]
            h3t = edgeconv(s, 2, h2t[0])       # [128,N]
            h4t = edgeconv(s, 3, h3t[0])       # 2x [128,N]
            cat = [h1[0], h2t[0], h3t[0], h4t[0], h4t[1], onesrow]
            rows = [64, 64, 128, 128, 128, 1]
            # ---- conv5 (channel-major) + fused max/mean pooling over n
            for eb in range(8):
                esl = slice(eb * 128, (eb + 1) * 128)
                for half in range(2):
                    nsl = slice(half * 512, (half + 1) * 512)
                    p5 = ps_m.tile([128, 512], f32, tag="mm")
                    for kki in range(6):
                        nc.tensor.matmul(
                            p5[:], lhsT=w5sb[kki][0:rows[kki], esl],
                            rhs=cat[kki][0:rows[kki], nsl],
                            start=(kki == 0), stop=(kki == 5))
                    h5sb = wpool.tile([128, 512], f32, tag="h5sb")
                    nc.scalar.activation(h5sb[:], p5[:], AF.Prelu, alpha=NEG,
                                         accum_out=sums[:, eb + 8 * half, s:s+1])
                    nc.vector.pool_max(maxs[:, eb + 8 * half, s:s+1], h5sb[:])
            for eb in range(8):
                nc.vector.tensor_max(pooled2[:, eb, s:s+1],
                                     maxs[:, eb, s:s+1], maxs[:, eb + 8, s:s+1])
                nc.vector.tensor_add(pooled2[:, eb + 8, s:s+1],
                                     sums[:, eb, s:s+1], sums[:, eb + 8, s:s+1])

        # ---- FC head (both samples at once, N=SPC)
        z1 = cpool.tile([128, 4, SPC], f32, tag="z1")
        for mb in range(4):
            zp = ps_t.tile([128, SPC], f32, tag="tp")
            for kk in range(16):
                lt = wpool.tile([128, 128], f32, tag="l1jit", bufs=4)
                nc.sync.dma_start(
                    lt[:], l1_d.ap()[kk * 128:(kk + 1) * 128,
                                     mb * 128:(mb + 1) * 128])
                nc.tensor.matmul(zp[:], lhsT=lt[:], rhs=pooled2[:, kk, :],
                                 start=(kk == 0), stop=(kk == 15))
            nc.scalar.activation(z1[:, mb, :], zp[:], AF.Prelu,
                                 bias=b6[:, mb:mb+1], alpha=NEG)
        z2 = cpool.tile([128, 2, SPC], f32, tag="z2")
        for mb in range(2):
            zp = ps_t.tile([128, SPC], f32, tag="tp")
            for kk in range(4):
                nc.tensor.matmul(zp[:], lhsT=l2sb[kk][:, mb * 128:(mb + 1) * 128],
                                 rhs=z1[:, kk, :], start=(kk == 0), stop=(kk == 3))
            nc.scalar.activation(z2[:, mb, :], zp[:], AF.Prelu,
                                 bias=b7[:, mb:mb+1], alpha=NEG)
        zp = ps_t.tile([40, SPC], f32, tag="tp")
        for kk in range(2):
            nc.tensor.matmul(zp[:], lhsT=l3sb[kk][:], rhs=z2[:, kk, :],
                             start=(kk == 0), stop=(kk == 1))
        osb = cpool.tile([40, SPC], f32, tag="osb")
        nc.scalar.activation(osb[:], zp[:], AF.Identity, bias=b3[:])
        nc.sync.dma_start(out_d.ap().rearrange("s o -> o s"), osb[:])

        for _p in (ps_t, ps_m, ps_d, dram_pool, gpool, wpool, dpool, hpool, cpool):
            _p.release()

    nc.compile()
    return nc


def _prep_weights(inputs):
    """Host-side folding of BN scales/biases into matmul operands."""
    inp = {k: np.asarray(v) for k, v in inputs.items()}
    rs = np.float32(1.0 / np.sqrt(1.0 + EPS))
    maps = {}
    for i, (w, g, b) in enumerate([("W1", "g1", "b1"), ("W2", "g2", "b2"),
                                   ("W3", "g3", "b3"), ("W4", "g4", "b4")]):
        W, g, b = inp[w], inp[g], inp[b]
        C = W.shape[1] // 2
        scale = (g * rs).astype(np.float32)
        Wd = W[:, :C] * scale[:, None]
        We = (W[:, C:] - W[:, :C]) * scale[:, None]
        maps[f"ru{i+1}"] = np.ascontiguousarray(Wd.T)
        maps[f"rvw{i+1}"] = np.ascontiguousarray(We.T)
        maps[f"rvb{i+1}"] = np.ascontiguousarray(b[None, :])
    s5 = (inp["g5"] * rs).astype(np.float32)
    w5 = (inp["W5"] * s5[:, None]).astype(np.float32)          # (1024, 512)
    w5t = np.concatenate([w5.T, inp["b5"][None, :]], axis=0)   # (513, 1024)
    maps["w5t"] = np.ascontiguousarray(w5t.astype(np.float32))
    s6 = (inp["g6"] * rs).astype(np.float32)
    l1 = (inp["L1"] * s6[:, None]).astype(np.float32)          # (512, 2048)
    l1[:, 1024:] *= np.float32(1.0 / N)                        # fold mean divisor
    maps["l1t"] = np.ascontiguousarray(l1.T)                   # (2048, 512)
    maps["b6v"] = np.ascontiguousarray(inp["b6"].reshape(4, 128).T)
    s7 = (inp["g7"] * rs).astype(np.float32)
    l2 = (inp["L2"] * s7[:, None]).astype(np.float32)
    maps["l2t"] = np.ascontiguousarray(l2.T)                   # (512, 256)
    b7v = (s7 * inp["l2b"] + inp["b7"]).astype(np.float32)
    maps["b7v"] = np.ascontiguousarray(b7v.reshape(2, 128).T)
    maps["l3t"] = np.ascontiguousarray(inp["L3"].T.astype(np.float32))  # (256,40)
    maps["b3v"] = np.ascontiguousarray(inp["l3b"].reshape(40, 1).astype(np.float32))
    maps["axsb"] = np.array([[-1.0, 0.0], [0.0, -1.0]], dtype=np.float32)
    maps["rxsb"] = np.array([[0.0, 1.0], [1.0, 0.0]], dtype=np.float32)
    maps["idf"] = np.eye(128, dtype=np.float32)
    return maps


def kernel(**inputs):
    from concourse.bass_utils import run_bass_kernel_spmd

    if "nc" not in _cache:
        _cache["nc"] = _build_program()
    nc = _cache["nc"]

    wmaps = _prep_weights(inputs)
    x = np.asarray(inputs["x"], dtype=np.float32)  # (B, N, 3)
    in_maps = []
    for c in range(NCORES):
        xs = x[c * SPC:(c + 1) * SPC]                     # (SPC, N, 3)
        m = dict(wmaps)
        m["xt"] = np.ascontiguousarray(xs.transpose(0, 2, 1))  # (SPC, 3, N)
        in_maps.append(m)

    res = run_bass_kernel_spmd(nc, in_maps, core_ids=list(range(NCORES)))
    out = np.concatenate([res.results[c]["out"] for c in range(NCORES)], axis=0)
    return out.astype(np.float32)


if __name__ == "__main__":
    import reference  # only when run manually inside /root/problem
    inputs = reference.setup_inputs()
    out = kernel(**{k: np.asarray(v) for k, v in inputs.items()})
    print(out.shape, out.dtype)



# revision 17
# speedup vs baseline: 810.2283x; 810.2283x over previous
"""DGCNN classifier Trainium2 kernel (Bass/Tile), data-parallel over batch on 8 cores.

Reformulation (per EdgeConv layer with weight W=[Wd|Wc], BN scale g, bias b):
    pre(n,j) = W @ [h_j - h_n; h_n] = Wd h_j + (Wc-Wd) h_n
    BN scale s>0 and LReLU are monotone, so they commute with the neighbor max:
    h'(n) = LReLU( max_{j in knn(n)} (s*Wd h_j) + (s*(Wc-Wd) h_n + b) )
Per layer: pairwise -dist^2 matrix D from augmented PE matmuls, exact top-20
per row via DVE max8/max_index/match_replace, gather of uT = (s*Wd) h
(channel-major) via gpsimd ap_gather from SBUF, neighbor max via DVE strided
tensor_reduce, +vT and LReLU on ACT. Features stay channel-major [C, N]
throughout (no transposes; the four layer outputs are conv5's K-tiles).
"""

import numpy as np

K = 20
EPS = 1e-5
NEG = 0.2
N = 1024
B = 16
NCORES = 8
SPC = B // NCORES  # samples per core
CS = [3, 64, 64, 128]      # in-channels per edgeconv layer
OS = [64, 64, 128, 256]    # out-channels per edgeconv layer
NEGINF = -1.0e30

_cache = {}


def _build_program():
    import concourse.mybir as mybir
    from concourse import bacc
    import concourse.tile as tile

    f32 = mybir.dt.float32
    u32 = mybir.dt.uint32
    i16 = mybir.dt.int16
    AF = mybir.ActivationFunctionType
    ALU = mybir.AluOpType
    AX = mybir.AxisListType

    nc = bacc.Bacc("TRN2", target_bir_lowering=False, debug=False,
                   enable_asserts=False, num_devices=NCORES)

    # ---- DRAM I/O -------------------------------------------------------
    xt_d = nc.dram_tensor("xt", (SPC, 3, N), f32, kind="ExternalInput")
    ru_d = [nc.dram_tensor(f"ru{i+1}", (CS[i], OS[i]), f32, kind="ExternalInput")
            for i in range(4)]
    rvw_d = [nc.dram_tensor(f"rvw{i+1}", (CS[i], OS[i]), f32, kind="ExternalInput")
             for i in range(4)]
    rvb_d = [nc.dram_tensor(f"rvb{i+1}", (1, OS[i]), f32, kind="ExternalInput")
             for i in range(4)]
    w5_d = nc.dram_tensor("w5t", (513, N), f32, kind="ExternalInput")
    l1_d = nc.dram_tensor("l1t", (2048, 512), f32, kind="ExternalInput")
    b6_d = nc.dram_tensor("b6v", (128, 4), f32, kind="ExternalInput")
    l2_d = nc.dram_tensor("l2t", (512, 256), f32, kind="ExternalInput")
    b7_d = nc.dram_tensor("b7v", (128, 2), f32, kind="ExternalInput")
    l3_d = nc.dram_tensor("l3t", (256, 40), f32, kind="ExternalInput")
    b3_d = nc.dram_tensor("b3v", (40, 1), f32, kind="ExternalInput")
    axsb_d = nc.dram_tensor("axsb", (2, 2), f32, kind="ExternalInput")
    rxsb_d = nc.dram_tensor("rxsb", (2, 2), f32, kind="ExternalInput")
    idf_d = nc.dram_tensor("idf", (128, 128), f32, kind="ExternalInput")
    out_d = nc.dram_tensor("out", (SPC, 40), f32, kind="ExternalOutput")

    with tile.TileContext(nc) as tc:
        cpool = tc.alloc_tile_pool(name="const", bufs=1)
        hpool = tc.alloc_tile_pool(name="hts", bufs=1)
        dpool = tc.alloc_tile_pool(name="dsb", bufs=2)
        wpool = tc.alloc_tile_pool(name="work", bufs=2)
        gpool = tc.alloc_tile_pool(name="gath", bufs=2)
        gupool = tc.alloc_tile_pool(name="gu", bufs=1)
        dram_pool = tc.alloc_tile_pool(name="wdram", bufs=1, space="DRAM")
        ps_d = tc.alloc_tile_pool(name="ps_d", bufs=2, space="PSUM")
        ps_m = tc.alloc_tile_pool(name="ps_m", bufs=2, space="PSUM")
        ps_t = tc.alloc_tile_pool(name="ps_t", bufs=2, space="PSUM")

        def load_const(ap, tag):
            t = cpool.tile(list(ap.shape), ap.dtype, tag=tag)
            nc.sync.dma_start(t[:], ap[:])
            return t

        ru = [load_const(ru_d[i].ap(), f"ru{i}") for i in range(4)]
        rvw = [load_const(rvw_d[i].ap(), f"rvw{i}") for i in range(4)]
        rvb = [load_const(rvb_d[i].ap(), f"rvb{i}") for i in range(4)]
        idf = load_const(idf_d.ap(), "idf")
        b6 = load_const(b6_d.ap(), "b6")
        b7 = load_const(b7_d.ap(), "b7")
        b3 = load_const(b3_d.ap(), "b3")
        axsb = load_const(axsb_d.ap(), "axsb")
        rxsb = load_const(rxsb_d.ap(), "rxsb")
        # w5t split into K-tiles matching [h1,h2,h3,h4a,h4b,ones]
        w5sb = []
        w5rows = [(0, 64), (64, 128), (128, 256), (256, 384), (384, 512), (512, 513)]
        for i, (r0, r1) in enumerate(w5rows):
            t = cpool.tile([r1 - r0, N], f32, tag=f"w5_{i}")
            nc.sync.dma_start(t[:], w5_d.ap()[r0:r1, :])
            w5sb.append(t)
        l2sb = []
        for kk in range(4):
            t = cpool.tile([128, 256], f32, tag=f"l2_{kk}")
            nc.sync.dma_start(t[:], l2_d.ap()[kk * 128:(kk + 1) * 128, :])
            l2sb.append(t)
        l3sb = []
        for kk in range(2):
            t = cpool.tile([128, 40], f32, tag=f"l3_{kk}")
            nc.sync.dma_start(t[:], l3_d.ap()[kk * 128:(kk + 1) * 128, :])
            l3sb.append(t)

        onescol = cpool.tile([128, 2], f32, tag="onescol")
        nc.gpsimd.memset(onescol[:], 1.0)
        onesrow = cpool.tile([1, N], f32, tag="onesrow")
        nc.gpsimd.memset(onesrow[:], 1.0)



        pooled2 = cpool.tile([128, 16, SPC], f32, tag="pooled2")
        maxs = cpool.tile([128, 16, SPC], f32, tag="maxs")
        sums = cpool.tile([128, 16, SPC], f32, tag="sums")

        def edgeconv(s, L, hT, hrep=None):
            """EdgeConv layer, channel-major. hT: [C, N] tile. Returns list of
            <=128-row [*, N] tiles holding the transposed output features.
            For L1, hrep is the input replicated on all 8 core groups so each
            gpsimd core gathers only its own 16-point group (320 idxs)."""
            C, O = CS[L], OS[L]
            nblk = (O + 127) // 128
            # ---- sq(n) = sum_c hT(c,n)^2, duplicated on partitions 0-1
            h2 = wpool.tile([C, N], f32, tag="h2sq")
            nc.scalar.activation(h2[:], hT[0:C, :], AF.Square)
            sqps = []
            for half in range(2):
                sqp = ps_m.tile([128, 512], f32, tag="mm")
                nc.tensor.matmul(sqp[0:2, :], lhsT=onescol[0:C, 0:2],
                                 rhs=h2[0:C, half * 512:(half + 1) * 512],
                                 start=True, stop=True)
                sqps.append(sqp)
            # ---- A = [2hT; -sq; -1], R = [hT; 1; sq]
            Ah = wpool.tile([C, N], f32, tag="Ah")
            nc.scalar.activation(Ah[:], hT[0:C, :], AF.Copy, scale=2.0)
            Ax = wpool.tile([2, N], f32, tag="Ax")
            Rx = wpool.tile([2, N], f32, tag="Rx")
            for half in range(2):
                nsl = slice(half * 512, (half + 1) * 512)
                nc.scalar.activation(Ax[0:2, nsl], sqps[half][0:2, :], AF.Identity,
                                     scale=axsb[:, 0:1], bias=axsb[:, 1:2])
                nc.scalar.activation(Rx[0:2, nsl], sqps[half][0:2, :], AF.Identity,
                                     scale=rxsb[:, 0:1], bias=rxsb[:, 1:2])
            # ---- uT, vT (channel-major). For L4 (O=256) u is computed
            # post-gather from the gathered input features instead (halves
            # the gpsimd gather volume), so uts is skipped there.
            gather_h = (L == 3)
            uts, vts = [], []
            for ot in range(nblk):
                oc = min(128, O - ot * 128)
                osl = slice(ot * 128, ot * 128 + oc)
                vt = wpool.tile([128, N], f32, tag=f"vt{ot}")
                ut = None
                if not gather_h:
                    ut = wpool.tile([128, N], f32, tag=f"ut{ot}")
                for half in range(2):
                    nsl = slice(half * 512, (half + 1) * 512)
                    if not gather_h:
                        up = ps_m.tile([128, 512], f32, tag="mm")
                        nc.tensor.matmul(up[0:oc, :], lhsT=ru[L][:, osl],
                                         rhs=hT[0:C, nsl], start=True, stop=True)
                        nc.scalar.copy(ut[0:oc, nsl], up[0:oc, :])
                    vp = ps_m.tile([128, 512], f32, tag="mm")
                    nc.tensor.matmul(vp[0:oc, :], lhsT=rvw[L][:, osl],
                                     rhs=hT[0:C, nsl], start=True, stop=False)
                    nc.tensor.matmul(vp[0:oc, :], lhsT=rvb[L][:, osl],
                                     rhs=onesrow[:, nsl], start=False, stop=True)
                    nc.scalar.copy(vt[0:oc, nsl], vp[0:oc, :])
                uts.append(ut)
                vts.append(vt)

            hT_new = []
            for ot in range(nblk):
                hT_new.append(hpool.tile([128, N], f32, tag=f"hT_{s}_{L}_{ot}", name=f"hT_{s}_{L}_{ot}"))

            # ---- per-block: D, top-20, marshal wrapped list into W_all
            if L > 0:
                wall = wpool.tile([16, 8 * 160], i16, tag="wall")
            for b in range(8):
                msl = slice(b * 128, (b + 1) * 128)
                Dp = ps_d.tile([128, N], f32, tag="Dp")
                for half in range(2):
                    nsl = slice(half * 512, (half + 1) * 512)
                    nc.tensor.matmul(Dp[:, nsl], lhsT=Ah[0:C, msl],
                                     rhs=hT[0:C, nsl], start=True, stop=False)
                    nc.tensor.matmul(Dp[:, nsl], lhsT=Ax[:, msl],
                                     rhs=Rx[:, nsl], start=False, stop=True)
                Dsb = dpool.tile([128, N], f32, tag="Dsb")
                vals = wpool.tile([128, 24], f32, tag="vals")
                idx = wpool.tile([128, 24], u32, tag="idx")
                nc.vector.max(vals[:, 0:8], Dp[:])
                nc.vector.max_index(idx[:, 0:8], vals[:, 0:8], Dp[:])
                nc.vector.match_replace(Dsb[:], vals[:, 0:8], Dp[:], NEGINF)
                nc.vector.max(vals[:, 8:16], Dsb[:])
                nc.vector.max_index(idx[:, 8:16], vals[:, 8:16], Dsb[:])
                nc.vector.match_replace(Dsb[:], vals[:, 8:16], Dsb[:], NEGINF)
                nc.vector.max(vals[:, 16:24], Dsb[:])
                nc.vector.max_index(idx[:, 16:24], vals[:, 16:24], Dsb[:])
                if L == 0:
                    # mosaic gather: partition p's idx rows ARE point p's
                    # neighbor list, so each core gathers its own 16-point
                    # group (320 idxs) from the replicated 3-channel input.
                    tb = wpool.tile([128, 20], i16, tag="tb")
                    nc.vector.tensor_copy(tb[:], idx[:, 0:20])
                    g1 = gpool.tile([128, 320], f32, tag="g1")
                    nc.gpsimd.ap_gather(
                        out_ap=g1[:, :], in_ap=hrep[:, :],
                        idxs_ap=tb[:, :], channels=128, num_elems=N, d=1,
                        num_idxs=320)
                    gx = gpool.tile([128, 2560], f32, tag="g")
                    for k8 in range(8):
                        nc.sync.dma_start(
                            gx[0:C, :].rearrange("c (s p) -> c s p", s=20)[
                                :, :, 16 * k8:16 * k8 + 16],
                            g1[16 * k8:16 * k8 + C, :].rearrange(
                                "c (s t) -> c s t", s=20))
                    gu = gupool.tile([128, 2560], f32, tag="gu")
                    for ch in range(5):
                        csl = slice(ch * 512, (ch + 1) * 512)
                        up = ps_m.tile([128, 512], f32, tag="mm")
                        nc.tensor.matmul(up[0:64, :], lhsT=ru[0][:, :],
                                         rhs=gx[0:C, csl], start=True,
                                         stop=True)
                        nc.scalar.copy(gu[0:64, csl], up[0:64, :])
                    acc = wpool.tile([128, 128], f32, tag="acc")
                    nc.vector.tensor_reduce(
                        acc[0:64, :],
                        gu[0:64, :].rearrange("o (s p) -> o p s", s=20),
                        axis=AX.X, op=ALU.max)
                    nc.vector.tensor_add(acc[0:64, :], acc[0:64, :],
                                         vts[0][0:64, msl])
                    nc.scalar.activation(hT_new[0][0:64, msl], acc[0:64, :],
                                         AF.Prelu, alpha=NEG)
                    continue
                # marshal: wrapped list W[q, s*8+pp] = idx[16*pp+q, s]
                idxf = wpool.tile([128, 24], f32, tag="idxf")
                nc.vector.tensor_copy(idxf[:], idx[:])
                t1p = ps_t.tile([24, 128], f32, tag="tp")
                nc.tensor.transpose(t1p[:], idxf[:], idf[:])
                t1s = wpool.tile([24, 128], f32, tag="t1s")
                nc.scalar.copy(t1s[:], t1p[:])
                wp = ps_t.tile([16, 160], f32, tag="tp")
                for pp in range(8):
                    nc.tensor.transpose(wp[0:16, pp * 20:(pp + 1) * 20],
                                        t1s[0:20, pp * 16:(pp + 1) * 16],
                                        idf[0:20, 0:20])
                nc.scalar.copy(
                    wall[:, b * 160:(b + 1) * 160].rearrange(
                        "q (s pp) -> q pp s", s=20),
                    wp[:].rearrange("q (pp s) -> q pp s", pp=8))

            if L == 0:
                return hT_new

            # ---- replicate wrapped lists to all 8 core groups (DRAM bounce)
            wd = dram_pool.tile([16, 8 * 160], i16, tag="wd")
            nc.sync.dma_start(wd[:], wall[:])
            wrep = wpool.tile([128, 8 * 160], i16, tag="wrep")
            for r8 in range(8):
                nc.sync.dma_start(wrep[r8 * 16:(r8 + 1) * 16, :], wd[:])

            # ---- gather + neighbor-max + vT + LReLU
            for b in range(8):
                msl = slice(b * 128, (b + 1) * 128)
                if gather_h:
                    # one gather of the C=128 input features per block, then
                    # matmul the gathered columns up to u-space (O channels)
                    g = gpool.tile([128, 2560], f32, tag="g")
                    nc.gpsimd.ap_gather(
                        out_ap=g[:, :], in_ap=hT[0:C, :],
                        idxs_ap=wrep[:, b * 160:(b + 1) * 160],
                        channels=128, num_elems=N, d=1, num_idxs=2560)
                    for ot in range(nblk):
                        osl = slice(ot * 128, (ot + 1) * 128)
                        gu = gupool.tile([128, 2560], f32, tag="gu")
                        for ch in range(5):
                            csl = slice(ch * 512, (ch + 1) * 512)
                            up = ps_m.tile([128, 512], f32, tag="mm")
                            nc.tensor.matmul(up[:], lhsT=ru[L][:, osl],
                                             rhs=g[:, csl], start=True,
                                             stop=True)
                            nc.scalar.copy(gu[:, csl], up[:])
                        acc = wpool.tile([128, 128], f32, tag="acc")
                        nc.vector.tensor_reduce(
                            acc[:], gu[:].rearrange("o (s p) -> o p s", s=20),
                            axis=AX.X, op=ALU.max)
                        nc.vector.tensor_add(acc[:], acc[:], vts[ot][:, msl])
                        nc.scalar.activation(hT_new[ot][:, msl], acc[:],
                                             AF.Prelu, alpha=NEG)
                    continue
                for ot in range(nblk):
                    oc = min(128, O - ot * 128)
                    g = gpool.tile([128, 2560], f32, tag="g")
                    nc.gpsimd.ap_gather(
                        out_ap=g[0:oc, :], in_ap=uts[ot][0:oc, :],
                        idxs_ap=wrep[0:oc, b * 160:(b + 1) * 160],
                        channels=oc, num_elems=N, d=1, num_idxs=2560)
                    acc = wpool.tile([128, 128], f32, tag="acc")
                    nc.vector.tensor_reduce(
                        acc[0:oc, :],
                        g[0:oc, :].rearrange("o (s p) -> o p s", s=20),
                        axis=AX.X, op=ALU.max)
                    nc.vector.tensor_add(acc[0:oc, :], acc[0:oc, :],
                                         vts[ot][0:oc, msl])
                    nc.scalar.activation(hT_new[ot][0:oc, msl], acc[0:oc, :],
                                         AF.Prelu, alpha=NEG)
            return hT_new

        for s in range(SPC):
            x_sb = hpool.tile([3, N], f32, tag=f"x_{s}")
            nc.sync.dma_start(x_sb[:], xt_d.ap()[s])
            hrep = hpool.tile([128, N], f32, tag="hrep")
            nc.vector.memset(hrep[:], 0.0)
            for k8 in range(8):
                nc.sync.dma_start(hrep[16 * k8:16 * k8 + 3, :], xt_d.ap()[s])
            h1 = edgeconv(s, 0, x_sb, hrep=hrep)  # [64,N]
            h2t = edgeconv(s, 1, h1[0])        # [64,N]
            h3t = edgeconv(s, 2, h2t[0])       # [128,N]
            h4t = edgeconv(s, 3, h3t[0])       # 2x [128,N]
            cat = [h1[0], h2t[0], h3t[0], h4t[0], h4t[1], onesrow]
            rows = [64, 64, 128, 128, 128, 1]
            # ---- conv5 (channel-major) + fused max/mean pooling over n
            for eb in range(8):
                esl = slice(eb * 128, (eb + 1) * 128)
                for half in range(2):
                    nsl = slice(half * 512, (half + 1) * 512)
                    p5 = ps_m.tile([128, 512], f32, tag="mm")
                    for kki in range(6):
                        nc.tensor.matmul(
                            p5[:], lhsT=w5sb[kki][0:rows[kki], esl],
                            rhs=cat[kki][0:rows[kki], nsl],
                            start=(kki == 0), stop=(kki == 5))
                    h5sb = wpool.tile([128, 512], f32, tag="h5sb")
                    nc.scalar.activation(h5sb[:], p5[:], AF.Prelu, alpha=NEG,
                                         accum_out=sums[:, eb + 8 * half, s:s+1])
                    nc.vector.pool_max(maxs[:, eb + 8 * half, s:s+1], h5sb[:])
            for eb in range(8):
                nc.vector.tensor_max(pooled2[:, eb, s:s+1],
                                     maxs[:, eb, s:s+1], maxs[:, eb + 8, s:s+1])
                nc.vector.tensor_add(pooled2[:, eb + 8, s:s+1],
                                     sums[:, eb, s:s+1], sums[:, eb + 8, s:s+1])

        # ---- FC head (both samples at once, N=SPC)
        z1 = cpool.tile([128, 4, SPC], f32, tag="z1")
        for mb in range(4):
            zp = ps_t.tile([128, SPC], f32, tag="tp")
            for kk in range(16):
                lt = wpool.tile([128, 128], f32, tag="l1jit", bufs=4)
                nc.sync.dma_start(
                    lt[:], l1_d.ap()[kk * 128:(kk + 1) * 128,
                                     mb * 128:(mb + 1) * 128])
                nc.tensor.matmul(zp[:], lhsT=lt[:], rhs=pooled2[:, kk, :],
                                 start=(kk == 0), stop=(kk == 15))
            nc.scalar.activation(z1[:, mb, :], zp[:], AF.Prelu,
                                 bias=b6[:, mb:mb+1], alpha=NEG)
        z2 = cpool.tile([128, 2, SPC], f32, tag="z2")
        for mb in range(2):
            zp = ps_t.tile([128, SPC], f32, tag="tp")
            for kk in range(4):
                nc.tensor.matmul(zp[:], lhsT=l2sb[kk][:, mb * 128:(mb + 1) * 128],
                                 rhs=z1[:, kk, :], start=(kk == 0), stop=(kk == 3))
            nc.scalar.activation(z2[:, mb, :], zp[:], AF.Prelu,
                                 bias=b7[:, mb:mb+1], alpha=NEG)
        zp = ps_t.tile([40, SPC], f32, tag="tp")
        for kk in range(2):
            nc.tensor.matmul(zp[:], lhsT=l3sb[kk][:], rhs=z2[:, kk, :],
                             start=(kk == 0), stop=(kk == 1))
        osb = cpool.tile([40, SPC], f32, tag="osb")
        nc.scalar.activation(osb[:], zp[:], AF.Identity, bias=b3[:])
        nc.sync.dma_start(out_d.ap().rearrange("s o -> o s"), osb[:])

        for _p in (ps_t, ps_m, ps_d, dram_pool, gupool, gpool, wpool, dpool,
                   hpool, cpool):
            _p.release()

    nc.compile()
    return nc


def _prep_weights(inputs):
    """Host-side folding of BN scales/biases into matmul operands."""
    inp = {k: np.asarray(v) for k, v in inputs.items()}
    rs = np.float32(1.0 / np.sqrt(1.0 + EPS))
    maps = {}
    for i, (w, g, b) in enumerate([("W1", "g1", "b1"), ("W2", "g2", "b2"),
                                   ("W3", "g3", "b3"), ("W4", "g4", "b4")]):
        W, g, b = inp[w], inp[g], inp[b]
        C = W.shape[1] // 2
        scale = (g * rs).astype(np.float32)
        Wd = W[:, :C] * scale[:, None]
        We = (W[:, C:] - W[:, :C]) * scale[:, None]
        maps[f"ru{i+1}"] = np.ascontiguousarray(Wd.T)
        maps[f"rvw{i+1}"] = np.ascontiguousarray(We.T)
        maps[f"rvb{i+1}"] = np.ascontiguousarray(b[None, :])
    s5 = (inp["g5"] * rs).astype(np.float32)
    w5 = (inp["W5"] * s5[:, None]).astype(np.float32)          # (1024, 512)
    w5t = np.concatenate([w5.T, inp["b5"][None, :]], axis=0)   # (513, 1024)
    maps["w5t"] = np.ascontiguousarray(w5t.astype(np.float32))
    s6 = (inp["g6"] * rs).astype(np.float32)
    l1 = (inp["L1"] * s6[:, None]).astype(np.float32)          # (512, 2048)
    l1[:, 1024:] *= np.float32(1.0 / N)                        # fold mean divisor
    maps["l1t"] = np.ascontiguousarray(l1.T)                   # (2048, 512)
    maps["b6v"] = np.ascontiguousarray(inp["b6"].reshape(4, 128).T)
    s7 = (inp["g7"] * rs).astype(np.float32)
    l2 = (inp["L2"] * s7[:, None]).astype(np.float32)
    maps["l2t"] = np.ascontiguousarray(l2.T)                   # (512, 256)
    b7v = (s7 * inp["l2b"] + inp["b7"]).astype(np.float32)
    maps["b7v"] = np.ascontiguousarray(b7v.reshape(2, 128).T)
    maps["l3t"] = np.ascontiguousarray(inp["L3"].T.astype(np.float32))  # (256,40)
    maps["b3v"] = np.ascontiguousarray(inp["l3b"].reshape(40, 1).astype(np.float32))
    maps["axsb"] = np.array([[-1.0, 0.0], [0.0, -1.0]], dtype=np.float32)
    maps["rxsb"] = np.array([[0.0, 1.0], [1.0, 0.0]], dtype=np.float32)
    maps["idf"] = np.eye(128, dtype=np.float32)
    return maps


def _get_runner():
    """Build the Bass program + AOT-compiled 8-core PJRT executable ONCE and
    cache it. Per-call work is then: fold weights if changed (device-resident
    otherwise), ship x (~200KB), dispatch, fetch the (16, 40) output."""
    if "runner" in _cache:
        return _cache["runner"]

    import jax
    from jax.sharding import Mesh, PartitionSpec, NamedSharding
    from jax.experimental.shard_map import shard_map
    from concourse import bass2jax
    import concourse.mybir as mybir

    nc = _build_program()
    _cache["nc"] = nc
    bass2jax.install_neuronx_cc_hook()

    partition_name = nc.partition_id_tensor.name if nc.partition_id_tensor else None
    assert nc.dbg_addr is None
    in_specs = []          # (name, global_shape, dtype) for real params
    out_names, out_avals = [], []
    for alloc in nc.m.functions[0].allocations:
        if not isinstance(alloc, mybir.MemoryLocationSet):
            continue
        name = alloc.memorylocations[0].name
        shape = tuple(alloc.tensor_shape) if alloc.tensor_shape else None
        dtype = mybir.dt.np(alloc.dtype) if alloc.dtype is not None else None
        if alloc.kind == "ExternalInput":
            if name != partition_name:
                in_specs.append((name, (NCORES * shape[0],) + shape[1:], dtype))
        elif alloc.kind == "ExternalOutput":
            out_names.append(name)
            out_avals.append(jax.core.ShapedArray(shape, dtype))
    n_params = len(in_specs)
    n_outs = len(out_names)
    all_in_names = tuple(n for n, _, _ in in_specs) + tuple(out_names)
    if partition_name is not None:
        all_in_names = all_in_names + (partition_name,)
    donate = tuple(range(n_params, n_params + n_outs))

    devices = jax.devices()[:NCORES]
    mesh = Mesh(np.asarray(devices), ("core",))
    shd = NamedSharding(mesh, PartitionSpec("core"))

    def _body(*args):
        operands = list(args)
        if partition_name is not None:
            operands.append(bass2jax.partition_id_tensor())
        outs = bass2jax._bass_exec_p.bind(
            *operands,
            out_avals=tuple(out_avals),
            in_names=all_in_names,
            out_names=tuple(out_names),
            lowering_input_output_aliases=(),
            sim_require_finite=True,
            sim_require_nnan=True,
            nc=nc,
        )
        return tuple(outs)

    zero_out_shapes = [
        ((NCORES * a.shape[0],) + a.shape[1:], a.dtype) for a in out_avals
    ]
    structs = [jax.ShapeDtypeStruct(s, d, sharding=shd) for _, s, d in in_specs]
    structs += [jax.ShapeDtypeStruct(s, d, sharding=shd) for s, d in zero_out_shapes]

    def compile_fn():
        jitted = jax.jit(
            shard_map(_body, mesh=mesh,
                      in_specs=(PartitionSpec("core"),) * (n_params + n_outs),
                      out_specs=(PartitionSpec("core"),) * n_outs,
                      check_rep=False),
            donate_argnums=donate, keep_unused=True)
        return jitted.lower(*structs).compile()

    try:
        compiled = bass2jax.fast_dispatch_compile(compile_fn)
    except Exception:
        compiled = compile_fn()

    runner = {
        "compiled": compiled,
        "param_names": [n for n, _, _ in in_specs],
        "zero_out_shapes": zero_out_shapes,
        "shd": shd,
        "device_put": jax.device_put,
    }
    _cache["runner"] = runner
    return runner


def kernel(**inputs):
    runner = _get_runner()
    shd = runner["shd"]
    dput = runner["device_put"]

    # ---- weights: fold + upload only when they change (device-resident) ----
    wnames = [k for k in inputs if k != "x"]
    raw = {k: np.asarray(inputs[k]) for k in wnames}
    cached = _cache.get("raw_weights")
    if cached is None or any(
            k not in cached or not np.array_equal(cached[k], raw[k])
            for k in wnames):
        wmaps = _prep_weights(inputs)
        wdev = {}
        for name, m in wmaps.items():
            rep = np.broadcast_to(m[None], (NCORES,) + m.shape).reshape(
                (NCORES * m.shape[0],) + m.shape[1:])
            wdev[name] = dput(np.ascontiguousarray(rep), shd)
        _cache["raw_weights"] = raw
        _cache["wdev"] = wdev
    wdev = _cache["wdev"]

    # ---- x: (B, N, 3) -> (B, 3, N), sharded over batch ----
    x = np.asarray(inputs["x"], dtype=np.float32)
    xt = dput(np.ascontiguousarray(x.transpose(0, 2, 1)), shd)

    args = []
    for name in runner["param_names"]:
        args.append(xt if name == "xt" else wdev[name])
    for s, d in runner["zero_out_shapes"]:
        args.append(dput(np.zeros(s, d), shd))

    out_arrs = runner["compiled"](*args)
    return np.asarray(out_arrs[0]).astype(np.float32)


if __name__ == "__main__":
    import reference  # only when run manually inside /root/problem
    inputs = reference.setup_inputs()
    out = kernel(**{k: np.asarray(v) for k, v in inputs.items()})
    print(out.shape, out.dtype)

